# revision 21
# baseline (speedup 1.0000x reference)
"""Trainium2 Bass kernel for nn_CrossAttnMem (channel self-attention + batch-flattened
cross attention) — token-sharded SPMD with an on-device Gram AllReduce.

Both attention paths factor through rank-64 Gram matrices:
  self:  scores[b,h] = Wqu_h^T (Eu_b^T Eu_b) Wku_h
  cross: S[bl]       = Wq^T (El_bl^T Eu_bu) Wk   per bu block of the flattened K
so the only large contractions are (a) the Grams over N=4096 tokens and (b) the
final output matmuls emb_u @ M — both shard perfectly over tokens.

Sharding: core c owns tokens [c*512, (c+1)*512) of ALL 8 batch rows.
  phase 1  each core computes partial Grams over its tokens         (big, local)
  phase 2  AllReduce(320KB fp32) -> every core has the full Grams   (tiny, NeuronLink)
  phase 3  every core redundantly runs the small softmax algebra:
           scores -> InstanceNorm stats -> exp -> row-sums folded into the
           output projections -> per-pair [64,64] effective matrices (tiny)
  phase 4  out[:, my_tokens, :] = Eu[my tokens] @ M / Weff          (big, local)

Host <-> device I/O is the bottleneck in this harness (axon tunnel), so emb
ships as bf16 sharded by token (0.5MB/core), all weights are baked into the
NEFF as inline constants, output returns as bf16 (0.5MB/core, disjoint), and
the PJRT dispatch is traced once and cached with persistent device-side
output buffers.
"""

import numpy as np

H = 8
C = 64
HC = 512
N = 4096
B = 4
EPS = 1e-5
NCORES = 8
TT = 4                        # 128-token tiles per core
CNT_CROSS = float(HC * B * HC)
CNT_SELF = float(C * C)

_CACHE = {}


def _build(W_qu, W_ku, W_vu, W_ql2u, W_kl2u, W_vl2u, W_out_u, W_out_l2u):
    import concourse.mybir as mybir
    import concourse.tile as tile
    from concourse import bacc

    dt = mybir.dt
    f32, bf16 = dt.float32, dt.bfloat16
    AF = mybir.ActivationFunctionType

    nc = bacc.Bacc("TRN2", target_bir_lowering=False, debug=False,
                   num_devices=NCORES)

    e_d = nc.dram_tensor("e", [128, 2048], bf16, kind="ExternalInput").ap()
    oloc_d = nc.dram_tensor("oloc", [TT, 128, 512], bf16).ap()
    og_d = nc.dram_tensor("og", [NCORES, TT, 128, 512], bf16,
                          addr_space="Shared").ap()
    out_d = nc.dram_tensor("out", [NCORES, TT, 128, 512], bf16,
                           kind="ExternalOutput").ap()
    gpart_d = nc.dram_tensor("gpart", [64, 1280], f32).ap()
    gsum_d = nc.dram_tensor("gsum", [64, 1280], f32, addr_space="Shared").ap()

    # ---- constants baked into the NEFF (uploaded once at model load) ----
    ca = lambda a: np.ascontiguousarray(a, dtype=np.float32)
    w_ou = W_out_u.reshape(C, H, C)
    consts = {
        "wq": ca(W_ql2u),                                  # [64, 512]
        "wk": ca(W_kl2u),                                  # [64, 512]
        "wvT": ca(W_vl2u.T.reshape(4, 128, 64)
                  .transpose(1, 0, 2).reshape(128, 256)),  # c-chunk major
        "woq": ca(W_out_l2u.reshape(4, 128, 64)
                  .transpose(1, 0, 2).reshape(128, 256)),  # q-tile major
        "wqu": ca(W_qu),                                   # [64, 512]
        "wku": ca(W_ku),                                   # [64, 512]
        "wvut": ca(np.concatenate(
            [W_vu[:, h * 64:(h + 1) * 64].T for h in range(H)], axis=1)),
        "woup": ca(np.concatenate(
            [w_ou[:, h, :] for h in range(H)], axis=1)),
        "identf": ca(np.eye(128)),                         # [128, 128]
        "ones128": ca(np.ones((128, 1))),
        "onesr": ca(np.ones((1, 128))),
        "sel8": ca(np.kron(np.eye(2), np.ones((4, 1)))),   # [8, 2]
    }
    cd = {k: nc.inline_tensor(v, name=k).ap() for k, v in consts.items()}

    with tile.TileContext(nc) as tc:
        with (
            tc.tile_pool(name="const", bufs=1) as cst,
            tc.tile_pool(name="emb", bufs=1) as embp,
            tc.tile_pool(name="work", bufs=1) as wrk,
        ):
            def load(pool, ap, shape, dtype=f32):
                t = pool.tile(list(shape), dtype, name=f"L_{ap.tensor.name}",
                              tag=f"L_{ap.tensor.name}")
                nc.sync.dma_start(t[:], ap)
                return t

            e_sb = load(embp, e_d, (128, 2048), bf16)
            wq = load(cst, cd["wq"], (64, 512))
            wk = load(cst, cd["wk"], (64, 512))
            wvT = load(cst, cd["wvT"], (128, 256))
            woq = load(cst, cd["woq"], (128, 256))
            wqu = load(cst, cd["wqu"], (64, 512))
            wku = load(cst, cd["wku"], (64, 512))
            wvut = load(cst, cd["wvut"], (64, 512))
            woup = load(cst, cd["woup"], (64, 512))
            identf = load(cst, cd["identf"], (128, 128))
            ones128 = load(cst, cd["ones128"], (128, 1))
            onesr = load(cst, cd["onesr"], (1, 128))
            sel8 = load(cst, cd["sel8"], (8, 2))

            def esl(t, r):
                """e tile slice: token tile t, batch-row r -> [128, 64] bf16."""
                o = t * 512 + r * 64
                return e_sb[:, o:o + 64]

            # ------------- Phase 1: partial Grams over my 512 tokens -------------
            # gpack cols: bl*256 -> G[bl] = El_bl^T [Eu0|Eu1|Eu2|Eu3]
            #             1024 + b*64 -> Guu_b = Eu_b^T Eu_b
            gpack = wrk.tile([64, 1280], f32, tag="gpack")
            with tc.tile_pool(name="g1", bufs=1, space="PSUM") as g1:
                G_ps = g1.tile([64, 1280], f32)
                for bl in range(4):
                    for t in range(TT):
                        nc.tensor.matmul(
                            G_ps[:, bl * 256:(bl + 1) * 256],
                            esl(t, bl),
                            e_sb[:, t * 512 + 256:t * 512 + 512],
                            start=(t == 0), stop=(t == TT - 1))
                for b in range(4):
                    for t in range(TT):
                        sl = esl(t, 4 + b)
                        nc.tensor.matmul(
                            G_ps[:, 1024 + b * 64:1024 + (b + 1) * 64],
                            sl, sl, start=(t == 0), stop=(t == TT - 1))
                nc.scalar.copy(gpack[:], G_ps[:])

            # ------------- Phase 2: AllReduce the Grams -------------
            nc.sync.dma_start(gpart_d, gpack[:])
            nc.gpsimd.collective_compute(
                "AllReduce", mybir.AluOpType.add,
                replica_groups=[list(range(NCORES))],
                ins=[gpart_d], outs=[gsum_d])
            gsum = wrk.tile([64, 1280], f32, tag="gsum")
            nc.sync.dma_start(gsum[:], gsum_d)

            def G(bl, bu):
                return gsum[:, bl * 256 + bu * 64:bl * 256 + (bu + 1) * 64]

            def Guu(b):
                return gsum[:, 1024 + b * 64:1024 + (b + 1) * 64]

            # ------------- Phase 3a: cross-attention small algebra -------------
            # per bl: S = Wq^T G Wk (as [512q, 2048m]), inorm stats, exp,
            # rowsum -> fold into Wout rows, Y = E^T Wout', M[bl,bu] = Wv Y_bu
            M_all = wrk.tile([64, 1024], f32, tag="M_all")   # (bl, bu) [64,64]
            V1 = wrk.tile([64, 2048], f32, tag="V1")
            S_sb = wrk.tile([128, 8192], f32, tag="S_sb")
            E_sb = wrk.tile([128, 8192], f32, tag="E_sb")
            dump = wrk.tile([128, 2048], f32, tag="dump")
            for bl in range(4):
                with tc.tile_pool(name=f"v1p{bl}", bufs=1, space="PSUM") as v1p:
                    for bu in range(4):
                        V1_ps = v1p.tile([64, 512], f32, tag="v1ps")
                        nc.tensor.matmul(V1_ps[:], G(bl, bu), wq[:])
                        nc.scalar.copy(V1[:, bu * 512:(bu + 1) * 512], V1_ps[:])
                ssum = wrk.tile([128, 8], f32, tag="ssum")
                with tc.tile_pool(name=f"sp{bl}", bufs=1, space="PSUM") as sp:
                    for qt in range(4):
                        S_ps = sp.tile([128, 2048], f32, tag="S_ps")
                        for bu in range(4):
                            nc.tensor.matmul(
                                S_ps[:, bu * 512:(bu + 1) * 512],
                                V1[:, bu * 512 + qt * 128:bu * 512 + qt * 128 + 128],
                                wk[:])
                        nc.scalar.activation(
                            S_sb[:, qt * 2048:(qt + 1) * 2048], S_ps[:],
                            AF.Copy, accum_out=ssum[:, qt:qt + 1])
                        nc.scalar.activation(
                            dump[:], S_ps[:], AF.Square,
                            accum_out=ssum[:, 4 + qt:5 + qt])
                    # stats -> (scale, bias) broadcast over partitions
                    t8_ps = sp.tile([8, 1], f32, tag="t8")
                    nc.tensor.matmul(t8_ps[:], ssum[:], ones128[:])
                    t8 = wrk.tile([8, 1], f32, tag="t8sb")
                    nc.scalar.copy(t8[:], t8_ps[:])
                    st_ps = sp.tile([1, 2], f32, tag="st")
                    nc.tensor.matmul(st_ps[:], t8[:], sel8[:])
                    mean = wrk.tile([1, 1], f32, tag="c0")
                    ex2 = wrk.tile([1, 1], f32, tag="c1")
                    m2 = wrk.tile([1, 1], f32, tag="c2")
                    var = wrk.tile([1, 1], f32, tag="c3")
                    std = wrk.tile([1, 1], f32, tag="c4")
                    rstd = wrk.tile([1, 1], f32, tag="c5")
                    nb = wrk.tile([1, 1], f32, tag="c6")
                    pair = wrk.tile([1, 2], f32, tag="c7")
                    nc.scalar.mul(mean[:], st_ps[:, 0:1], 1.0 / CNT_CROSS)
                    nc.scalar.mul(ex2[:], st_ps[:, 1:2], 1.0 / CNT_CROSS)
                    nc.scalar.square(m2[:], mean[:])
                    nc.vector.tensor_sub(var[:], ex2[:], m2[:])
                    nc.vector.tensor_scalar_add(var[:], var[:], EPS)
                    nc.scalar.activation(std[:], var[:], AF.Sqrt)
                    nc.vector.reciprocal(rstd[:], std[:])
                    nc.vector.tensor_mul(nb[:], mean[:], rstd[:])
                    nc.scalar.copy(pair[:, 0:1], rstd[:])
                    nc.scalar.mul(pair[:, 1:2], nb[:], -1.0)
                    bc_ps = sp.tile([128, 2], f32, tag="bc")
                    nc.tensor.matmul(bc_ps[:], onesr[:], pair[:])
                    bcv = wrk.tile([128, 2], f32, tag="bcv")
                    nc.scalar.copy(bcv[:], bc_ps[:])
                # exp + per-row sums
                rs = wrk.tile([128, 4], f32, tag="rs")
                for qt in range(4):
                    nc.scalar.activation(
                        E_sb[:, qt * 2048:(qt + 1) * 2048],
                        S_sb[:, qt * 2048:(qt + 1) * 2048],
                        AF.Exp, scale=bcv[:, 0:1], bias=bcv[:, 1:2],
                        accum_out=rs[:, qt:qt + 1])
                rec = wrk.tile([128, 4], f32, tag="rec")
                nc.vector.reciprocal(rec[:], rs[:])
                woutp = wrk.tile([128, 256], f32, tag="woutp")
                for qt in range(4):
                    nc.vector.tensor_scalar_mul(
                        woutp[:, qt * 64:(qt + 1) * 64],
                        woq[:, qt * 64:(qt + 1) * 64], rec[:, qt:qt + 1])
                Y_sb = wrk.tile([128, 1024], f32, tag="Y_sb")
                with tc.tile_pool(name=f"yp{bl}", bufs=1, space="PSUM") as yp:
                    Y_ps = yp.tile([128, 1024], f32, tag="Y_ps")
                    for j in range(16):
                        for qt in range(4):
                            nc.tensor.matmul(
                                Y_ps[:, j * 64:(j + 1) * 64],
                                E_sb[:, qt * 2048 + j * 128:
                                     qt * 2048 + (j + 1) * 128],
                                woutp[:, qt * 64:(qt + 1) * 64],
                                start=(qt == 0), stop=(qt == 3))
                    nc.scalar.copy(Y_sb[:], Y_ps[:])
                with tc.tile_pool(name=f"mp{bl}", bufs=1, space="PSUM") as mp:
                    M_ps = mp.tile([64, 256], f32, tag="M_ps")
                    for bu in range(4):
                        for k in range(4):
                            j = bu * 4 + k
                            nc.tensor.matmul(
                                M_ps[:, bu * 64:(bu + 1) * 64],
                                wvT[:, k * 64:(k + 1) * 64],
                                Y_sb[:, j * 64:(j + 1) * 64],
                                start=(k == 0), stop=(k == 3))
                    nc.scalar.copy(M_all[:, bl * 256:(bl + 1) * 256], M_ps[:])

            # ------------- Phase 3b: self-attention small algebra -------------
            Weff = wrk.tile([64, 256], f32, tag="Weff")      # per b [64,64]
            for b in range(4):
                with tc.tile_pool(name=f"sf{b}", bufs=1, space="PSUM") as sf:
                    Ts_ps = sf.tile([64, 512], f32, tag="Ts")
                    nc.tensor.matmul(Ts_ps[:], Guu(b), wku[:])
                    Ts = wrk.tile([64, 512], f32, tag="Tssb")
                    nc.scalar.copy(Ts[:], Ts_ps[:])
                    sc_ps = sf.tile([64, 512], f32, tag="scps")
                    for h in range(H):
                        nc.tensor.matmul(
                            sc_ps[:, h * 64:(h + 1) * 64],
                            wqu[:, h * 64:(h + 1) * 64],
                            Ts[:, h * 64:(h + 1) * 64])
                    ss8 = wrk.tile([64, 16], f32, tag="ss8")
                    dmp = wrk.tile([64, 64], f32, tag="dmp")
                    for h in range(H):
                        blk = sc_ps[:, h * 64:(h + 1) * 64]
                        nc.scalar.activation(dmp[:], blk, AF.Copy,
                                             accum_out=ss8[:, h:h + 1])
                        nc.scalar.activation(dmp[:], blk, AF.Square,
                                             accum_out=ss8[:, 8 + h:9 + h])
                    tot_ps = sf.tile([8, 2], f32, tag="tot")
                    nc.tensor.matmul(tot_ps[:, 0:1], ss8[:, 0:8],
                                     ones128[0:64, :])
                    nc.tensor.matmul(tot_ps[:, 1:2], ss8[:, 8:16],
                                     ones128[0:64, :])
                    mean_s = wrk.tile([8, 1], f32, tag="m0")
                    ex2_s = wrk.tile([8, 1], f32, tag="m1")
                    m2_s = wrk.tile([8, 1], f32, tag="m2")
                    var_s = wrk.tile([8, 1], f32, tag="m3")
                    std_s = wrk.tile([8, 1], f32, tag="m4")
                    rstd_s = wrk.tile([8, 1], f32, tag="m5")
                    nb_s = wrk.tile([8, 1], f32, tag="m6")
                    pairs = wrk.tile([8, 2], f32, tag="m7")
                    nc.scalar.mul(mean_s[:], tot_ps[:, 0:1], 1.0 / CNT_SELF)
                    nc.scalar.mul(ex2_s[:], tot_ps[:, 1:2], 1.0 / CNT_SELF)
                    nc.scalar.square(m2_s[:], mean_s[:])
                    nc.vector.tensor_sub(var_s[:], ex2_s[:], m2_s[:])
                    nc.vector.tensor_scalar_add(var_s[:], var_s[:], EPS)
                    nc.scalar.activation(std_s[:], var_s[:], AF.Sqrt)
                    nc.vector.reciprocal(rstd_s[:], std_s[:])
                    nc.vector.tensor_mul(nb_s[:], mean_s[:], rstd_s[:])
                    nc.scalar.copy(pairs[:, 0:1], rstd_s[:])
                    nc.scalar.mul(pairs[:, 1:2], nb_s[:], -1.0)
                    rT_ps = sf.tile([1, 8], f32, tag="rT")
                    bT_ps = sf.tile([1, 8], f32, tag="bT")
                    nc.tensor.transpose(rT_ps[:], pairs[:, 0:1],
                                        identf[0:8, 0:8])
                    nc.tensor.transpose(bT_ps[:], pairs[:, 1:2],
                                        identf[0:8, 0:8])
                    rnT = wrk.tile([1, 16], f32, tag="rnT")
                    nc.scalar.copy(rnT[:, 0:8], rT_ps[:])
                    nc.scalar.copy(rnT[:, 8:16], bT_ps[:])
                    sbm_ps = sf.tile([64, 16], f32, tag="sbm")
                    nc.tensor.matmul(sbm_ps[:], onesr[0:1, 0:64], rnT[:])
                    sbm = wrk.tile([64, 16], f32, tag="sbmsb")
                    nc.scalar.copy(sbm[:], sbm_ps[:])
                    Es = wrk.tile([64, 512], f32, tag="Es")
                    er = wrk.tile([64, 8], f32, tag="er")
                    for h in range(H):
                        nc.scalar.activation(
                            Es[:, h * 64:(h + 1) * 64],
                            sc_ps[:, h * 64:(h + 1) * 64], AF.Exp,
                            scale=sbm[:, h:h + 1], bias=sbm[:, 8 + h:9 + h],
                            accum_out=er[:, h:h + 1])
                    rec_er = wrk.tile([64, 8], f32, tag="rec_er")
                    nc.vector.reciprocal(rec_er[:], er[:])
                    wosc = wrk.tile([64, 512], f32, tag="wosc")
                    for h in range(H):
                        nc.vector.tensor_scalar_mul(
                            wosc[:, h * 64:(h + 1) * 64],
                            woup[:, h * 64:(h + 1) * 64], rec_er[:, h:h + 1])
                    Ys_ps = sf.tile([64, 512], f32, tag="Ys")
                    for h in range(H):
                        nc.tensor.matmul(
                            Ys_ps[:, h * 64:(h + 1) * 64],
                            Es[:, h * 64:(h + 1) * 64],
                            wosc[:, h * 64:(h + 1) * 64])
                    Ys = wrk.tile([64, 512], f32, tag="Yssb")
                    nc.scalar.copy(Ys[:], Ys_ps[:])
                    We_ps = sf.tile([64, 64], f32, tag="Weps")
                    for h in range(H):
                        nc.tensor.matmul(We_ps[:], wvut[:, h * 64:(h + 1) * 64],
                                         Ys[:, h * 64:(h + 1) * 64],
                                         start=(h == 0), stop=(h == H - 1))
                    nc.scalar.copy(Weff[:, b * 64:(b + 1) * 64], We_ps[:])

            # ------------- Phase 4: outputs for my 512 tokens -------------
            # upcast Eu tiles to f32 once (transpose out dtype must match in)
            eu32 = wrk.tile([128, 1024], f32, tag="eu32")    # (t, b) [128,64]
            for t in range(TT):
                for b in range(4):
                    nc.scalar.copy(eu32[:, (t * 4 + b) * 64:(t * 4 + b + 1) * 64],
                                   esl(t, 4 + b))
            with (
                tc.tile_pool(name="op", bufs=2, space="PSUM") as op,
                tc.tile_pool(name="osb", bufs=2) as osbp,
            ):
                for t in range(TT):
                    TP_ps = op.tile([64, 512], f32, tag="TP")
                    for b in range(4):
                        nc.tensor.transpose(
                            TP_ps[:, b * 128:(b + 1) * 128],
                            eu32[:, (t * 4 + b) * 64:(t * 4 + b + 1) * 64],
                            identf[:])
                    etr = osbp.tile([64, 512], f32, tag="etr")
                    nc.scalar.copy(etr[:], TP_ps[:])
                    O_ps = op.tile([128, 512], f32, tag="O")
                    for bl in range(4):
                        for bu in range(4):
                            nc.tensor.matmul(
                                O_ps[:, bl * 64:(bl + 1) * 64],
                                etr[:, bu * 128:(bu + 1) * 128],
                                M_all[:, bl * 256 + bu * 64:
                                      bl * 256 + (bu + 1) * 64],
                                start=(bu == 0), stop=(bu == 3))
                    for b in range(4):
                        nc.tensor.matmul(
                            O_ps[:, 256 + b * 64:256 + (b + 1) * 64],
                            etr[:, b * 128:(b + 1) * 128],
                            Weff[:, b * 64:(b + 1) * 64])
                    ob = osbp.tile([128, 512], bf16, tag="ob")
                    nc.scalar.copy(ob[:], O_ps[:])
                    nc.sync.dma_start(oloc_d[t], ob[:])
            # gather every core's token-slice so the host fetches ONE shard
            nc.gpsimd.collective_compute(
                "AllGather", mybir.AluOpType.bypass,
                replica_groups=[list(range(NCORES))],
                ins=[oloc_d], outs=[og_d])
            nc.sync.dma_start(out_d, og_d)
    nc.compile()
    return nc


def _make_dispatch(nc):
    import concourse.mybir as mybir
    from concourse.bass2jax import (_bass_exec_p, partition_id_tensor,
                                    install_neuronx_cc_hook)
    import jax
    from jax.sharding import Mesh, PartitionSpec, NamedSharding
    from jax.experimental.shard_map import shard_map

    install_neuronx_cc_hook()
    partition_name = (nc.partition_id_tensor.name
                      if nc.partition_id_tensor else None)
    in_names, out_names, out_avals, zero_outs = [], [], [], []
    for alloc in nc.m.functions[0].allocations:
        if not isinstance(alloc, mybir.MemoryLocationSet):
            continue
        name = alloc.memorylocations[0].name
        if alloc.kind == "ExternalInput":
            if name != partition_name:
                in_names.append(name)
        elif alloc.kind == "ExternalOutput":
            out_names.append(name)
            shape = tuple(alloc.tensor_shape)
            dtype = mybir.dt.np(alloc.dtype)
            out_avals.append(jax.core.ShapedArray(shape, dtype))
            zero_outs.append(np.zeros(shape, dtype))
    n_params = len(in_names)
    n_outs = len(out_avals)
    all_in_names = list(in_names) + out_names
    if partition_name is not None:
        all_in_names.append(partition_name)

    def _body(*args):
        operands = list(args)
        if partition_name is not None:
            operands.append(partition_id_tensor())
        outs = _bass_exec_p.bind(
            *operands,
            out_avals=tuple(out_avals),
            in_names=tuple(all_in_names),
            out_names=tuple(out_names),
            lowering_input_output_aliases=(),
            sim_require_finite=True,
            sim_require_nnan=True,
            nc=nc,
        )
        return tuple(outs)

    devices = jax.devices()[:NCORES]
    mesh = Mesh(np.asarray(devices), ("core",))
    # the output is AllGather-replicated on device; fetch one shard only
    in_specs = (PartitionSpec("core"),) * n_params + (PartitionSpec(),) * n_outs
    out_specs = (PartitionSpec(),) * len(out_names)
    sharded = jax.jit(
        shard_map(_body, mesh=mesh, in_specs=in_specs, out_specs=out_specs,
                  check_rep=False),
        keep_unused=True,
    )
    zeros_dev = [
        jax.device_put(z, NamedSharding(mesh, PartitionSpec()))
        for z in zero_outs
    ]
    return sharded, zeros_dev


def _weights_key(ws):
    import hashlib
    h = hashlib.blake2b(digest_size=16)
    for w in ws:
        h.update(np.ascontiguousarray(w, np.float32).tobytes())
    return h.hexdigest()


def _get_runner(ws):
    key = _weights_key(ws)
    if _CACHE.get("key") != key:
        nc = _build(*[np.asarray(w, np.float32) for w in ws])
        sharded, zeros_dev = _make_dispatch(nc)
        _CACHE.update(key=key, nc=nc, sharded=sharded, zeros_dev=zeros_dev)
        # warm once so jit tracing + neuronxcc compile are paid at build time
        import jax
        import ml_dtypes
        dummy = np.zeros((NCORES * 128, 2048), ml_dtypes.bfloat16)
        jax.block_until_ready(sharded(dummy, *zeros_dev))
    return _CACHE["sharded"], _CACHE["zeros_dev"]


def _prep_e(emb):
    """[8, 4096, 64] fp32 -> bf16 [8 cores * 128, t*512 + r*64 + ch]."""
    import ml_dtypes
    eb = np.asarray(emb, np.float32).astype(ml_dtypes.bfloat16)
    return np.ascontiguousarray(
        eb.reshape(8, NCORES, TT, 128, 64)
        .transpose(1, 3, 2, 0, 4).reshape(NCORES * 128, 2048))


def _finish(out_arr):
    """[NCORES, TT, 128, 512] bf16 -> [8, 4096, 64] fp32."""
    o = np.asarray(out_arr).reshape(NCORES, TT, 128, H, 64)
    return np.ascontiguousarray(
        o.transpose(3, 0, 1, 2, 4).reshape(8, 4096, 64).astype(np.float32))


def kernel(emb, pseudo_label, pseudo_prob_map, W_qu, W_ku, W_vu, W_ql2u,
           W_kl2u, W_vl2u, W_out_u, W_out_l2u, using_SMem, **_unused):
    del pseudo_label, pseudo_prob_map, using_SMem
    ws = (W_qu, W_ku, W_vu, W_ql2u, W_kl2u, W_vl2u, W_out_u, W_out_l2u)
    sharded, zeros_dev = _get_runner(ws)
    e = _prep_e(emb)
    out = sharded(e, *zeros_dev)
    return _finish(out[0])
